# revision 33
# baseline (speedup 1.0000x reference)
"""BlockSparseLocallyConnected forward on 8 Trainium2 NeuronCores.

Window-column shard: core k owns output columns nc in {4k..4k+3}, all 64
batches.  The PE does the real MACs:

  out[b, nr, nc] = sum_{dr,dc} xpad[b, 16nr+dr, 16nc+dc] * w[nr*32+nc, dr*32+dc]

Contraction (dr, dc) is split into 8 chunks q=(qr, hc) of 128 = (dr_local 8,
c16 16); SBUF partition p = 16*dr_local + c16 holds x rows r = dr_local
(mod 8), cols c = c16 (mod 16) -- window columns start at multiples of 16,
so ONE copy of x serves every (nc, hc) with a pure free-dim offset.  Rows
are stored per partition as [m', b, par, idx] with r = 16*idx + 8*par +
dr_local, so the moving AP for window-row nr_x is contiguous (stride 1).

Per (nc_local j, q): lhsT = weights [128, 32 nr_w] (stationary), rhs = x
[128, (b 16, nr_x 32) = 512] (moving), accumulated over the 8 q-chunks into
PSUM[32j:32j+32, 512] via tile_position=(0, 32j).  j rotates innermost so
consecutive MMs land on different PE col-groups, which execute CONCURRENTLY
(128x32 col-tiling).  The matmul computes all (nr_w, nr_x) cross terms;
only the diagonal nr_w == nr_x is the real output.

The kernel is bound by TOTAL per-core HBM bytes, so bytes are minimized
three ways: (1) odd 8-row groups (par=1) of x ship as fp8e4m3 and feed the
matmuls directly against bf16 weights; even groups stay bf16 (rel err
~1.9e-2, under the 2e-2 gate).  (2) The diagonal is extracted ON-CHIP: a
DVE mask-multiply (mask[q, b*32+x] = (x == q%32), shipped once on the
gpsimd ring) + segmented tensor_reduce turn each PSUM pass into a [128,16]
f32 tile, so the output DMA is 8KB/fq instead of 128KB of cross terms.
(3) The PE warmup burst (memset-fed) is front-loaded so HAM un-throttles
during the DMA fill instead of delaying the real stream.  Each (fq, dtype)
x slab is one contiguous ~0.7MB DMA on a single ring in exact consumption
order; w + mask + bias ride the gpsimd ring in parallel.
"""

import sys

sys.path.insert(0, "/opt/trn_rl_repo")

import numpy as np
import ml_dtypes

# ---- problem constants (hardcoded; kernel.py must be self-contained) ----
B = 64            # batch
H = W = 512
PH = PW = 8
FULL = 528        # padded H/W
NKH = NKW = 32    # window grid
NCORES = 8
NCL = 4           # window-columns per core
FQ = 4            # f-dim chunks (16 batches each)
BFQ = B // FQ     # 16
M = 5             # 16-col blocks per core span (80 cols)

BF16 = ml_dtypes.bfloat16
F8 = ml_dtypes.float8_e4m3fn

_CACHE = {}

TRACE = False          # test.py sets True to get exec_time_ns
LAST_RESULTS = None    # BassKernelResults of last run (for test.py)


def _build_program():
    import concourse.bass as bass
    import concourse.bacc as bacc
    import concourse.tile as tile
    from concourse import mybir

    dt_c = mybir.dt.bfloat16
    f32 = mybir.dt.float32
    dt8 = mybir.dt.float8e4

    nc = bacc.Bacc(
        "TRN2", target_bir_lowering=False, debug=False, num_devices=NCORES
    )
    xs = nc.dram_tensor("xs", [FQ, 128, M, BFQ, 33], dt_c,
                        kind="ExternalInput")
    x8 = nc.dram_tensor("x8", [FQ, 128, M, BFQ, 33], dt8,
                        kind="ExternalInput")
    # weights + diag mask + bias in ONE tensor with clean 2114B/partition
    # lines: [p, 1024 w | 32 mask | 1 bias].  A separate [128,1] bias DMA
    # emits 128 four-byte descriptors that clog the ring for ~1kcy; bf16
    # bias costs ~2^-9 relative on a 0.01-magnitude term -- negligible.
    # mask: mk[p, x] = (x == p % 32), broadcast over b via a stride-0 AP.
    wp = nc.dram_tensor("wp", [128, 1024 + 32 + 1], dt_c,
                        kind="ExternalInput")
    out_d = nc.dram_tensor("out", [128, FQ, BFQ], f32, kind="ExternalOutput")

    with tile.TileContext(nc) as tc:
        with (
            tc.tile_pool(name="xpool", bufs=FQ) as xpool,
            tc.tile_pool(name="cst", bufs=1) as cst,
            tc.tile_pool(name="wpsum", bufs=1, space="PSUM") as wpsum_pool,
            tc.tile_pool(name="psum", bufs=4, space="PSUM") as psum,
            tc.tile_pool(name="opool", bufs=2) as opool,
        ):
            warm = cst.tile([128, 512], dt_c)
            w_sb = cst.tile([128, 1024 + 32 + 1], dt_c, name="w")
            red = cst.tile([128, FQ, BFQ], f32, name="red")
            m_big = cst.tile([128, BFQ // 2, 32], dt_c, name="mbig")
            m_sb = w_sb[:, 1024:1056]
            b_sb = w_sb[:, 1056:1057]
            x_sb = [[None, None] for _ in range(FQ)]
            for fq in range(FQ):
                x_sb[fq][0] = xpool.tile(
                    [128, M, BFQ, 33], dt_c, tag="xb16", name=f"xb16_{fq}"
                )
                x_sb[fq][1] = xpool.tile(
                    [128, M, BFQ, 33], dt8, tag="xb8", name=f"xb8_{fq}"
                )

            # PE warmup burst FIRST: memset-fed matmuls during the DMA fill
            # window so HAM un-throttles (needs ~4096 busy cycles) before
            # the real stream -- front-loading the memset keeps the PE
            # queue from delaying real matmuls behind a late warmup block.
            nc.gpsimd.memset(warm[:], 1.0)
            wpsum = wpsum_pool.tile([128, 512], f32, tag="warm")
            for _ in range(8):
                nc.tensor.matmul(wpsum[:], warm[:, 0:128], warm[:],
                                 start=True, stop=True)

            # consts ship FIRST on the SAME sync ring, serialized ahead of
            # x: a concurrent second ring (SWDGE or qAct HWDGE) costs the
            # x stream ~2.3kcy of interference for 271KB, while pure
            # serialization costs only ~0.9kcy of delay.
            nc.sync.dma_start(out=w_sb[:], in_=wp[:])

            # Expand the mask for GPSIMD's half of the last-fq extraction
            # (broadcast APs stay on DVE), and preload the ACT function
            # table so the tail evac doesn't pay the 1.3us ACT_TABLE_LOAD.
            mk_b = bass.AP(
                tensor=m_sb.tensor, offset=m_sb.offset,
                ap=[list(m_sb.ap[0]), [0, BFQ // 2], [1, 32]],
            )
            nc.vector.tensor_copy(out=m_big[:], in_=mk_b)
            dumm = cst.tile([1, 1], f32)
            nc.scalar.activation(
                out=dumm[:], in_=w_sb[0:1, 0:1],
                func=mybir.ActivationFunctionType.Identity,
                bias=0.0, scale=1.0,
            )

            # The last fq's fp8 slab ships in two m-chunks: the 10 matmuls
            # reading m-blocks 0-2 start on the first chunk's (much
            # earlier) completion receipt; only 6 matmuls wait for the
            # final receipt.
            for fq in range(FQ):
                nc.sync.dma_start(out=x_sb[fq][0][:], in_=xs[fq])
                if fq < FQ - 1:
                    nc.sync.dma_start(out=x_sb[fq][1][:], in_=x8[fq])
                else:
                    nc.sync.dma_start(out=x_sb[fq][1][:, 0:3], in_=x8[fq, :, 0:3])
                    nc.sync.dma_start(out=x_sb[fq][1][:, 3:M], in_=x8[fq, :, 3:M])

            # Real stream: per fq, 8 q-chunks x 4 j = 32 matmuls of f=512
            # into one PSUM bank.  j innermost: consecutive MMs hit
            # different PE col-groups, which run CONCURRENTLY (128x32
            # col-tiling mode).  All input DMAs ride ONE ring (sync) in
            # exact consumption order.  par0 (bf16) chunks first: their
            # slab lands before the fp8 one.
            seq = [(j, qr, hc) for qr in (0, 2, 1, 3) for hc in range(2)
                   for j in range(NCL)]
            # last fq: within the fp8 half, m-chunk-2 readers (j+hc >= 3)
            # go last so only they wait on the final slab receipt.
            seq_l = ([t for t in seq if not (t[1] & 1)]
                     + [t for t in seq if (t[1] & 1) and t[0] + t[2] < 3]
                     + [t for t in seq if (t[1] & 1) and t[0] + t[2] >= 3])
            for fq in range(FQ):
                ps = psum.tile([128, 512], f32, tag="acc", name=f"acc{fq}")
                seen = [0] * NCL
                for j, qr, hc in (seq_l if fq == FQ - 1 else seq):
                    xt = x_sb[fq][qr & 1][:]
                    rhs = bass.AP(
                        tensor=xt.tensor,
                        offset=(xt.offset + 528 * (j + hc) + (qr >> 1)),
                        ap=[
                            list(xt.ap[0]),  # partition
                            [33, BFQ],       # b
                            [1, 32],         # nr_x
                        ],
                    )
                    wb = ((j * 4 + qr) * 2 + hc) * 32
                    nc.tensor.matmul(
                        ps[32 * j: 32 * j + 32, :],
                        w_sb[:, wb:wb + 32],
                        rhs,
                        start=(seen[j] == 0),
                        stop=(seen[j] == 7),
                        tile_position=(0, 32 * j),
                        # CoreSim's zero-region tracker is bank-granular and
                        # false-positives on 4 concurrent col-tiled groups.
                        skip_group_check=True,
                    )
                    seen[j] += 1
                # Diag extraction on DVE: tmp = (ps + bias) * mask (bias
                # folded in -- sum_x mask == 1 per (p, b) so the bias
                # survives the segment-sum exactly once), then segment-sum
                # the 32-wide nr_x groups -> red[:, fq] f32.  Only one
                # nonzero per segment, so no accumulation error.  The last
                # fq splits into b-halves to shorten the critical tail.
                psv = ps[:]
                halves = 2 if fq == FQ - 1 else 1
                hb = BFQ // halves
                tmps = []
                for h in range(halves):
                    tmp = opool.tile([128, hb, 32], dt_c, tag=f"tmp{h}",
                                     name=f"tmp{fq}_{h}")
                    tmps.append(tmp)
                    ps3 = bass.AP(
                        tensor=psv.tensor, offset=psv.offset + 32 * hb * h,
                        ap=[list(psv.ap[0]), [32, hb], [1, 32]],
                    )
                    if h == 0:
                        mk3 = bass.AP(
                            tensor=m_sb.tensor, offset=m_sb.offset,
                            ap=[list(m_sb.ap[0]), [0, hb], [1, 32]],
                        )
                        nc.vector.scalar_tensor_tensor(
                            out=tmp[:], in0=ps3,
                            scalar=b_sb, in1=mk3,
                            op0=mybir.AluOpType.add,
                            op1=mybir.AluOpType.mult,
                        )
                    else:
                        # last fq, half 1: ACT evacuates PSUM (+bias),
                        # GPSIMD mask-multiplies from SBUF -- both run in
                        # parallel with DVE's half-0 chain, cutting the
                        # serialized DVE tail from ~1.9k to ~1.5kcy.
                        ev1 = opool.tile([128, hb, 32], dt_c, tag="ev1",
                                         name="ev1")
                        nc.scalar.activation(
                            out=ev1[:], in_=ps3,
                            func=mybir.ActivationFunctionType.Identity,
                            bias=b_sb, scale=1.0,
                        )
                        nc.gpsimd.tensor_mul(out=tmp[:], in0=ev1[:],
                                             in1=m_big[:])
                for h in range(halves):
                    nc.vector.tensor_reduce(
                        out=red[:, fq, hb * h:hb * h + hb], in_=tmps[h][:],
                        axis=mybir.AxisListType.X,
                        op=mybir.AluOpType.add,
                    )
                # out DMAs: fq0-2 merged into one deferred 24KB DMA, the
                # last fq alone (8KB) so its receipt is the only tail.
                if fq == FQ - 2:
                    nc.scalar.dma_start(out=out_d[:, 0:FQ - 1],
                                        in_=red[:, 0:FQ - 1])
                elif fq == FQ - 1:
                    nc.scalar.dma_start(out=out_d[:, FQ - 1:FQ],
                                        in_=red[:, FQ - 1:FQ])
    nc.compile()
    return nc


def _prep_inputs(x, weight, bias):
    """Host-side packing into the transposed (mod-8 row, mod-16 col)
    partition layout; bf16 cast.  Returns per-core in_maps."""
    x = np.asarray(x, dtype=np.float32)
    weight = np.asarray(weight, dtype=np.float32)
    bias = np.asarray(bias, dtype=np.float32)

    xpad = np.zeros((B, FULL, FULL), dtype=np.float32)
    xpad[:, PH:PH + H, PW:PW + W] = x[:, 0]
    xpb = xpad.astype(BF16)

    # r = 16*idx + 8*par + dl
    dl = np.arange(8)[:, None, None]
    par = np.arange(2)[None, :, None]
    idx = np.arange(33)[None, None, :]
    r_map = 16 * idx + 8 * par + dl                      # [8, 2, 33]

    w4 = weight.reshape(32, 32, 32, 32)                  # [nr, nc, dr, dc]
    bv = bias.reshape(32, 32)                            # [nr, nc]

    # diag mask: mk[p, x] = (x == p % 32)
    mkv = (np.arange(32)[None, :]
           == (np.arange(128) % 32)[:, None]).astype(BF16)

    in_maps = []
    for k in range(NCORES):
        c_map = (16 * (4 * k + np.arange(M))[:, None]
                 + np.arange(16)[None, :])               # [m, c16]
        # gather -> [b, dl, par, idx, m, c16]
        g = xpb[:, r_map.reshape(8, 2, 33, 1, 1),
                c_map.reshape(1, 1, 1, M, 16)]
        # -> [fq, bi, dl, par, idx, m, c16]
        g = g.reshape(FQ, BFQ, 8, 2, 33, M, 16)
        # -> [fq, par, dl, c16, m, bi, idx]
        g = g.transpose(0, 3, 2, 6, 5, 1, 4)
        g = g.reshape(FQ, 2, 128, M, BFQ, 33)
        xsv = np.ascontiguousarray(g[:, 0])
        x8v = np.ascontiguousarray(g[:, 1]).astype(F8)

        # weights: [nr, j, qr, dl, hc, c16] -> [dl, c16, j, qr, hc, nr]
        wk = w4[:, 4 * k:4 * k + NCL].reshape(32, NCL, 4, 8, 2, 16)
        wk = wk.transpose(3, 5, 1, 2, 4, 0).reshape(128, 1024).astype(BF16)
        # bias: partition 32j + nr_w -> bias[nr_w, 4k+j]
        bk = bv[:, 4 * k:4 * k + NCL].T.reshape(128, 1).astype(BF16)
        wpk = np.ascontiguousarray(np.concatenate([wk, mkv, bk], axis=1))

        in_maps.append({"xs": xsv, "x8": x8v, "wp": wpk})
    return in_maps


def kernel(x, weight, bias):
    global LAST_RESULTS
    from concourse.bass_utils import run_bass_kernel_spmd

    if "nc" not in _CACHE:
        _CACHE["nc"] = _build_program()
    nc = _CACHE["nc"]

    in_maps = _prep_inputs(x, weight, bias)
    res = run_bass_kernel_spmd(
        nc, in_maps, core_ids=list(range(NCORES)), trace=TRACE
    )
    LAST_RESULTS = res

    out = np.empty((B, NKH, NKW), dtype=np.float32)
    for k in range(NCORES):
        # out[32j + nr, fq, bi] -> out[16fq + bi, nr, 4k + j]
        r4 = res.results[k]["out"].reshape(NCL, 32, FQ, BFQ)
        d = r4.transpose(2, 3, 1, 0)            # [fq, bi, nr, j]
        out[:, :, 4 * k:4 * k + NCL] = d.reshape(B, NKH, NCL)
    return out


# revision 36
# speedup vs baseline: 1.0249x; 1.0249x over previous
"""BlockSparseLocallyConnected forward on 8 Trainium2 NeuronCores.

Window-column shard: core k owns output columns nc in {4k..4k+3}, all 64
batches.  The PE does the real MACs:

  out[b, nr, nc] = sum_{dr,dc} xpad[b, 16nr+dr, 16nc+dc] * w[nr*32+nc, dr*32+dc]

Contraction (dr, dc) is split into 8 chunks q=(qr, hc) of 128 = (dr_local 8,
c16 16); SBUF partition p = 16*dr_local + c16 holds x rows r = dr_local
(mod 8), cols c = c16 (mod 16) -- window columns start at multiples of 16,
so ONE copy of x serves every (nc, hc) with a pure free-dim offset.  Rows
are stored per partition as [m', b, par, idx] with r = 16*idx + 8*par +
dr_local, so the moving AP for window-row nr_x is contiguous (stride 1).

Per (nc_local j, q): lhsT = weights [128, 32 nr_w] (stationary), rhs = x
[128, (b 16, nr_x 32) = 512] (moving), accumulated over the 8 q-chunks into
PSUM[32j:32j+32, 512] via tile_position=(0, 32j).  j rotates innermost so
consecutive MMs land on different PE col-groups, which execute CONCURRENTLY
(128x32 col-tiling).  The matmul computes all (nr_w, nr_x) cross terms;
only the diagonal nr_w == nr_x is the real output.

The kernel is bound by TOTAL per-core HBM bytes, so bytes are minimized
three ways: (1) odd 8-row groups (par=1) of x ship as fp8e4m3 and feed the
matmuls directly against bf16 weights; even groups stay bf16 (rel err
~1.9e-2, under the 2e-2 gate).  (2) The diagonal is extracted ON-CHIP: a
DVE mask-multiply (mask[q, b*32+x] = (x == q%32), shipped once on the
gpsimd ring) + segmented tensor_reduce turn each PSUM pass into a [128,16]
f32 tile, so the output DMA is 8KB/fq instead of 128KB of cross terms.
(3) The PE warmup burst (memset-fed) is front-loaded so HAM un-throttles
during the DMA fill instead of delaying the real stream.  Each (fq, dtype)
x slab is one contiguous ~0.7MB DMA on a single ring in exact consumption
order; w + mask + bias ride the gpsimd ring in parallel.
"""

import sys

sys.path.insert(0, "/opt/trn_rl_repo")

import numpy as np
import ml_dtypes

# ---- problem constants (hardcoded; kernel.py must be self-contained) ----
B = 64            # batch
H = W = 512
PH = PW = 8
FULL = 528        # padded H/W
NKH = NKW = 32    # window grid
NCORES = 8
NCL = 4           # window-columns per core
FQ = 4            # f-dim chunks (16 batches each)
BFQ = B // FQ     # 16
M = 5             # 16-col blocks per core span (80 cols)

BF16 = ml_dtypes.bfloat16
F8 = ml_dtypes.float8_e4m3fn

_CACHE = {}

TRACE = False          # test.py sets True to get exec_time_ns
LAST_RESULTS = None    # BassKernelResults of last run (for test.py)


def _build_program():
    import concourse.bass as bass
    import concourse.bacc as bacc
    import concourse.tile as tile
    from concourse import mybir

    dt_c = mybir.dt.bfloat16
    f32 = mybir.dt.float32
    dt8 = mybir.dt.float8e4

    nc = bacc.Bacc(
        "TRN2", target_bir_lowering=False, debug=False, num_devices=NCORES
    )
    xs = nc.dram_tensor("xs", [FQ, 128, M, BFQ, 33], dt_c,
                        kind="ExternalInput")
    x8 = nc.dram_tensor("x8", [FQ, 128, M, BFQ, 33], dt8,
                        kind="ExternalInput")
    # weights + diag mask + bias in ONE tensor with clean 2114B/partition
    # lines: [p, 1024 w | 32 mask | 1 bias].  A separate [128,1] bias DMA
    # emits 128 four-byte descriptors that clog the ring for ~1kcy; bf16
    # bias costs ~2^-9 relative on a 0.01-magnitude term -- negligible.
    # mask: mk[p, x] = (x == p % 32), broadcast over b via a stride-0 AP.
    wp = nc.dram_tensor("wp", [128, 1024 + 32 + 1], dt_c,
                        kind="ExternalInput")
    out_d = nc.dram_tensor("out", [128, FQ, BFQ], f32, kind="ExternalOutput")

    with tile.TileContext(nc) as tc:
        with (
            tc.tile_pool(name="xpool", bufs=FQ) as xpool,
            tc.tile_pool(name="cst", bufs=1) as cst,
            tc.tile_pool(name="wpsum", bufs=1, space="PSUM") as wpsum_pool,
            tc.tile_pool(name="psum", bufs=4, space="PSUM") as psum,
            tc.tile_pool(name="opool", bufs=2) as opool,
        ):
            warm = cst.tile([128, 512], dt_c)
            w_sb = cst.tile([128, 1024 + 32 + 1], dt_c, name="w")
            red = cst.tile([128, FQ, BFQ], f32, name="red")
            m_sb = w_sb[:, 1024:1056]
            b_sb = w_sb[:, 1056:1057]
            x_sb = [[None, None] for _ in range(FQ)]
            for fq in range(FQ):
                x_sb[fq][0] = xpool.tile(
                    [128, M, BFQ, 33], dt_c, tag="xb16", name=f"xb16_{fq}"
                )
                x_sb[fq][1] = xpool.tile(
                    [128, M, BFQ, 33], dt8, tag="xb8", name=f"xb8_{fq}"
                )

            # PE warmup burst FIRST: memset-fed matmuls during the DMA fill
            # window so HAM un-throttles (needs ~4096 busy cycles) before
            # the real stream -- front-loading the memset keeps the PE
            # queue from delaying real matmuls behind a late warmup block.
            nc.gpsimd.memset(warm[:], 1.0)
            wpsum = wpsum_pool.tile([128, 512], f32, tag="warm")
            for _ in range(8):
                nc.tensor.matmul(wpsum[:], warm[:, 0:128], warm[:],
                                 start=True, stop=True)

            # consts ship FIRST on the SAME sync ring, serialized ahead of
            # x: a concurrent second ring (SWDGE or qAct HWDGE) costs the
            # x stream ~2.3kcy of interference for 271KB, while pure
            # serialization costs only ~0.9kcy of delay.
            nc.sync.dma_start(out=w_sb[:], in_=wp[:])



            # The last fq's fp8 slab ships in two m-chunks: the 10 matmuls
            # reading m-blocks 0-2 start on the first chunk's (much
            # earlier) completion receipt; only 6 matmuls wait for the
            # final receipt.
            for fq in range(FQ):
                nc.sync.dma_start(out=x_sb[fq][0][:], in_=xs[fq])
                if fq < FQ - 1:
                    nc.sync.dma_start(out=x_sb[fq][1][:], in_=x8[fq])
                else:
                    nc.sync.dma_start(out=x_sb[fq][1][:, 0:3], in_=x8[fq, :, 0:3])
                    nc.sync.dma_start(out=x_sb[fq][1][:, 3:M], in_=x8[fq, :, 3:M])

            # Real stream: per fq, 8 q-chunks x 4 j = 32 matmuls of f=512
            # into one PSUM bank.  j innermost: consecutive MMs hit
            # different PE col-groups, which run CONCURRENTLY (128x32
            # col-tiling mode).  All input DMAs ride ONE ring (sync) in
            # exact consumption order.  par0 (bf16) chunks first: their
            # slab lands before the fp8 one.
            seq = [(j, qr, hc) for qr in (0, 2, 1, 3) for hc in range(2)
                   for j in range(NCL)]
            # last fq: within the fp8 half, m-chunk-2 readers (j+hc >= 3)
            # go last so only they wait on the final slab receipt.
            seq_l = ([t for t in seq if not (t[1] & 1)]
                     + [t for t in seq if (t[1] & 1) and t[0] + t[2] < 3]
                     + [t for t in seq if (t[1] & 1) and t[0] + t[2] >= 3])
            for fq in range(FQ):
                ps = psum.tile([128, 512], f32, tag="acc", name=f"acc{fq}")
                seen = [0] * NCL
                for j, qr, hc in (seq_l if fq == FQ - 1 else seq):
                    xt = x_sb[fq][qr & 1][:]
                    rhs = bass.AP(
                        tensor=xt.tensor,
                        offset=(xt.offset + 528 * (j + hc) + (qr >> 1)),
                        ap=[
                            list(xt.ap[0]),  # partition
                            [33, BFQ],       # b
                            [1, 32],         # nr_x
                        ],
                    )
                    wb = ((j * 4 + qr) * 2 + hc) * 32
                    nc.tensor.matmul(
                        ps[32 * j: 32 * j + 32, :],
                        w_sb[:, wb:wb + 32],
                        rhs,
                        start=(seen[j] == 0),
                        stop=(seen[j] == 7),
                        tile_position=(0, 32 * j),
                        # CoreSim's zero-region tracker is bank-granular and
                        # false-positives on 4 concurrent col-tiled groups.
                        skip_group_check=True,
                    )
                    seen[j] += 1
                # Diag extraction on DVE: tmp = (ps + bias) * mask (bias
                # folded in -- sum_x mask == 1 per (p, b) so the bias
                # survives the segment-sum exactly once), then segment-sum
                # the 32-wide nr_x groups -> red[:, fq] f32.  Only one
                # nonzero per segment, so no accumulation error.  The last
                # fq splits into b-halves to shorten the critical tail.
                psv = ps[:]
                halves = 2 if fq == FQ - 1 else 1
                hb = BFQ // halves
                tmps = []
                for h in range(halves):
                    tmp = opool.tile([128, hb, 32], dt_c, tag=f"tmp{h}",
                                     name=f"tmp{fq}_{h}")
                    tmps.append(tmp)
                    ps3 = bass.AP(
                        tensor=psv.tensor, offset=psv.offset + 32 * hb * h,
                        ap=[list(psv.ap[0]), [32, hb], [1, 32]],
                    )
                    mk3 = bass.AP(
                        tensor=m_sb.tensor, offset=m_sb.offset,
                        ap=[list(m_sb.ap[0]), [0, hb], [1, 32]],
                    )
                    nc.vector.scalar_tensor_tensor(
                        out=tmp[:], in0=ps3,
                        scalar=b_sb, in1=mk3,
                        op0=mybir.AluOpType.add, op1=mybir.AluOpType.mult,
                    )
                    nc.vector.tensor_reduce(
                        out=red[:, fq, hb * h:hb * h + hb], in_=tmp[:],
                        axis=mybir.AxisListType.X,
                        op=mybir.AluOpType.add,
                    )
                # out DMAs: fq0-2 merged into one deferred 24KB DMA, the
                # last fq alone (8KB) so its receipt is the only tail.
                if fq == FQ - 2:
                    nc.scalar.dma_start(out=out_d[:, 0:FQ - 1],
                                        in_=red[:, 0:FQ - 1])
                elif fq == FQ - 1:
                    nc.scalar.dma_start(out=out_d[:, FQ - 1:FQ],
                                        in_=red[:, FQ - 1:FQ])
    nc.compile()
    return nc


def _prep_inputs(x, weight, bias):
    """Host-side packing into the transposed (mod-8 row, mod-16 col)
    partition layout; bf16 cast.  Returns per-core in_maps."""
    x = np.asarray(x, dtype=np.float32)
    weight = np.asarray(weight, dtype=np.float32)
    bias = np.asarray(bias, dtype=np.float32)

    xpad = np.zeros((B, FULL, FULL), dtype=np.float32)
    xpad[:, PH:PH + H, PW:PW + W] = x[:, 0]
    xpb = xpad.astype(BF16)

    # r = 16*idx + 8*par + dl
    dl = np.arange(8)[:, None, None]
    par = np.arange(2)[None, :, None]
    idx = np.arange(33)[None, None, :]
    r_map = 16 * idx + 8 * par + dl                      # [8, 2, 33]

    w4 = weight.reshape(32, 32, 32, 32)                  # [nr, nc, dr, dc]
    bv = bias.reshape(32, 32)                            # [nr, nc]

    # diag mask: mk[p, x] = (x == p % 32)
    mkv = (np.arange(32)[None, :]
           == (np.arange(128) % 32)[:, None]).astype(BF16)

    in_maps = []
    for k in range(NCORES):
        c_map = (16 * (4 * k + np.arange(M))[:, None]
                 + np.arange(16)[None, :])               # [m, c16]
        # gather -> [b, dl, par, idx, m, c16]
        g = xpb[:, r_map.reshape(8, 2, 33, 1, 1),
                c_map.reshape(1, 1, 1, M, 16)]
        # -> [fq, bi, dl, par, idx, m, c16]
        g = g.reshape(FQ, BFQ, 8, 2, 33, M, 16)
        # -> [fq, par, dl, c16, m, bi, idx]
        g = g.transpose(0, 3, 2, 6, 5, 1, 4)
        g = g.reshape(FQ, 2, 128, M, BFQ, 33)
        xsv = np.ascontiguousarray(g[:, 0])
        x8v = np.ascontiguousarray(g[:, 1]).astype(F8)

        # weights: [nr, j, qr, dl, hc, c16] -> [dl, c16, j, qr, hc, nr]
        wk = w4[:, 4 * k:4 * k + NCL].reshape(32, NCL, 4, 8, 2, 16)
        wk = wk.transpose(3, 5, 1, 2, 4, 0).reshape(128, 1024).astype(BF16)
        # bias: partition 32j + nr_w -> bias[nr_w, 4k+j]
        bk = bv[:, 4 * k:4 * k + NCL].T.reshape(128, 1).astype(BF16)
        wpk = np.ascontiguousarray(np.concatenate([wk, mkv, bk], axis=1))

        in_maps.append({"xs": xsv, "x8": x8v, "wp": wpk})
    return in_maps


def kernel(x, weight, bias):
    global LAST_RESULTS
    from concourse.bass_utils import run_bass_kernel_spmd

    if "nc" not in _CACHE:
        _CACHE["nc"] = _build_program()
    nc = _CACHE["nc"]

    in_maps = _prep_inputs(x, weight, bias)
    res = run_bass_kernel_spmd(
        nc, in_maps, core_ids=list(range(NCORES)), trace=TRACE
    )
    LAST_RESULTS = res

    out = np.empty((B, NKH, NKW), dtype=np.float32)
    for k in range(NCORES):
        # out[32j + nr, fq, bi] -> out[16fq + bi, nr, 4k + j]
        r4 = res.results[k]["out"].reshape(NCL, 32, FQ, BFQ)
        d = r4.transpose(2, 3, 1, 0)            # [fq, bi, nr, j]
        out[:, :, 4 * k:4 * k + NCL] = d.reshape(B, NKH, NCL)
    return out


# revision 38
# speedup vs baseline: 1.1465x; 1.1186x over previous
"""BlockSparseLocallyConnected forward on 8 Trainium2 NeuronCores.

Window-column shard: core k owns output columns nc in {4k..4k+3}, all 64
batches.  The PE does the real MACs:

  out[b, nr, nc] = sum_{dr,dc} xpad[b, 16nr+dr, 16nc+dc] * w[nr*32+nc, dr*32+dc]

Contraction (dr, dc) is split into 8 chunks q=(qr, hc) of 128 = (dr_local 8,
c16 16); SBUF partition p = 16*dr_local + c16 holds x rows r = dr_local
(mod 8), cols c = c16 (mod 16) -- window columns start at multiples of 16,
so ONE copy of x serves every (nc, hc) with a pure free-dim offset.  Rows
are stored per partition as [m', b, par, idx] with r = 16*idx + 8*par +
dr_local, so the moving AP for window-row nr_x is contiguous (stride 1).

Per (nc_local j, q): lhsT = weights [128, 32 nr_w] (stationary), rhs = x
[128, (b 16, nr_x 32) = 512] (moving), accumulated over the 8 q-chunks into
PSUM[32j:32j+32, 512] via tile_position=(0, 32j).  j rotates innermost so
consecutive MMs land on different PE col-groups, which execute CONCURRENTLY
(128x32 col-tiling).  The matmul computes all (nr_w, nr_x) cross terms;
only the diagonal nr_w == nr_x is the real output.

The kernel is bound by TOTAL per-core HBM bytes, so bytes are minimized
three ways: (1) odd 8-row groups (par=1) of x ship as fp8e4m3 and feed the
matmuls directly against bf16 weights; even groups stay bf16 (rel err
~1.9e-2, under the 2e-2 gate).  (2) The diagonal is extracted ON-CHIP: a
DVE mask-multiply (mask[q, b*32+x] = (x == q%32), shipped once on the
gpsimd ring) + segmented tensor_reduce turn each PSUM pass into a [128,16]
f32 tile, so the output DMA is 8KB/fq instead of 128KB of cross terms.
(3) The PE warmup burst (memset-fed) is front-loaded so HAM un-throttles
during the DMA fill instead of delaying the real stream.  Each (fq, dtype)
x slab is one contiguous ~0.7MB DMA on a single ring in exact consumption
order; w + mask + bias ride the gpsimd ring in parallel.
"""

import sys

sys.path.insert(0, "/opt/trn_rl_repo")

import numpy as np
import ml_dtypes

# ---- problem constants (hardcoded; kernel.py must be self-contained) ----
B = 64            # batch
H = W = 512
PH = PW = 8
FULL = 528        # padded H/W
NKH = NKW = 32    # window grid
NCORES = 8
NCL = 4           # window-columns per core
FQ = 4            # f-dim chunks (16 batches each)
BFQ = B // FQ     # 16
M = 5             # 16-col blocks per core span (80 cols)

BF16 = ml_dtypes.bfloat16
F8 = ml_dtypes.float8_e4m3fn

_CACHE = {}

TRACE = False          # test.py sets True to get exec_time_ns
LAST_RESULTS = None    # BassKernelResults of last run (for test.py)


def _build_program():
    import concourse.bass as bass
    import concourse.bacc as bacc
    import concourse.tile as tile
    from concourse import mybir

    dt_c = mybir.dt.bfloat16
    f32 = mybir.dt.float32
    dt8 = mybir.dt.float8e4

    nc = bacc.Bacc(
        "TRN2", target_bir_lowering=False, debug=False, num_devices=NCORES
    )
    xs = nc.dram_tensor("xs", [FQ, 128, M, BFQ, 33], dt_c,
                        kind="ExternalInput")
    x8 = nc.dram_tensor("x8", [FQ, 128, M, BFQ, 33], dt8,
                        kind="ExternalInput")
    # weights + diag mask + bias in ONE tensor with clean 2114B/partition
    # lines: [p, 1024 w | 32 mask | 1 bias].  A separate [128,1] bias DMA
    # emits 128 four-byte descriptors that clog the ring for ~1kcy; bf16
    # bias costs ~2^-9 relative on a 0.01-magnitude term -- negligible.
    # mask: mk[p, x] = (x == p % 32), broadcast over b via a stride-0 AP.
    wp = nc.dram_tensor("wp", [128, 1024 + 32 + 1], dt_c,
                        kind="ExternalInput")
    out_d = nc.dram_tensor("out", [128, FQ, BFQ], f32, kind="ExternalOutput")

    with tile.TileContext(nc) as tc:
        with (
            tc.tile_pool(name="xpool", bufs=FQ) as xpool,
            tc.tile_pool(name="cst", bufs=1) as cst,
            tc.tile_pool(name="wpsum", bufs=1, space="PSUM") as wpsum_pool,
            tc.tile_pool(name="psum", bufs=4, space="PSUM") as psum,
            tc.tile_pool(name="opool", bufs=2) as opool,
        ):
            warm = cst.tile([128, 512], dt_c)
            w_sb = cst.tile([128, 1024 + 32 + 1], dt_c, name="w")
            red = cst.tile([128, FQ, BFQ], f32, name="red")
            m_sb = w_sb[:, 1024:1056]
            b_sb = w_sb[:, 1056:1057]
            x_sb = [[None, None] for _ in range(FQ)]
            for fq in range(FQ):
                x_sb[fq][0] = xpool.tile(
                    [128, M, BFQ, 33], dt_c, tag="xb16", name=f"xb16_{fq}"
                )
                x_sb[fq][1] = xpool.tile(
                    [128, M, BFQ, 33], dt8, tag="xb8", name=f"xb8_{fq}"
                )

            # PE warmup burst FIRST: memset-fed matmuls during the DMA fill
            # window so HAM un-throttles (needs ~4096 busy cycles) before
            # the real stream -- front-loading the memset keeps the PE
            # queue from delaying real matmuls behind a late warmup block.
            nc.gpsimd.memset(warm[:], 1.0)
            wpsum = wpsum_pool.tile([128, 512], f32, tag="warm")
            for _ in range(8):
                nc.tensor.matmul(wpsum[:], warm[:, 0:128], warm[:],
                                 start=True, stop=True)

            # consts ship FIRST on the SAME sync ring, serialized ahead of
            # x: a concurrent second ring (SWDGE or qAct HWDGE) costs the
            # x stream ~2.3kcy of interference for 271KB, while pure
            # serialization costs only ~0.9kcy of delay.
            nc.sync.dma_start(out=w_sb[:], in_=wp[:])



            for fq in range(FQ):
                nc.sync.dma_start(out=x_sb[fq][0][:], in_=xs[fq])
                nc.sync.dma_start(out=x_sb[fq][1][:], in_=x8[fq])

            # Real stream: per fq, 8 q-chunks x 4 j = 32 matmuls of f=512
            # into one PSUM bank.  j innermost: consecutive MMs hit
            # different PE col-groups, which run CONCURRENTLY (128x32
            # col-tiling mode).  All input DMAs ride ONE ring (sync) in
            # exact consumption order.  par0 (bf16) chunks first: their
            # slab lands before the fp8 one.
            seq = [(j, qr, hc) for qr in (0, 2, 1, 3) for hc in range(2)
                   for j in range(NCL)]
            for fq in range(FQ):
                ps = psum.tile([128, 512], f32, tag="acc", name=f"acc{fq}")
                seen = [0] * NCL
                for j, qr, hc in seq:
                    xt = x_sb[fq][qr & 1][:]
                    rhs = bass.AP(
                        tensor=xt.tensor,
                        offset=(xt.offset + 528 * (j + hc) + (qr >> 1)),
                        ap=[
                            list(xt.ap[0]),  # partition
                            [33, BFQ],       # b
                            [1, 32],         # nr_x
                        ],
                    )
                    wb = ((j * 4 + qr) * 2 + hc) * 32
                    nc.tensor.matmul(
                        ps[32 * j: 32 * j + 32, :],
                        w_sb[:, wb:wb + 32],
                        rhs,
                        start=(seen[j] == 0),
                        stop=(seen[j] == 7),
                        tile_position=(0, 32 * j),
                        # CoreSim's zero-region tracker is bank-granular and
                        # false-positives on 4 concurrent col-tiled groups.
                        skip_group_check=True,
                    )
                    seen[j] += 1
                # Diag extraction on DVE: tmp = (ps + bias) * mask (bias
                # folded in -- sum_x mask == 1 per (p, b) so the bias
                # survives the segment-sum exactly once), then segment-sum
                # the 32-wide nr_x groups -> red[:, fq] f32.  Only one
                # nonzero per segment, so no accumulation error.  The last
                # fq splits into b-halves to shorten the critical tail.
                psv = ps[:]
                halves = 2 if fq == FQ - 1 else 1
                hb = BFQ // halves
                tmps = []
                for h in range(halves):
                    tmp = opool.tile([128, hb, 32], dt_c, tag=f"tmp{h}",
                                     name=f"tmp{fq}_{h}")
                    tmps.append(tmp)
                    ps3 = bass.AP(
                        tensor=psv.tensor, offset=psv.offset + 32 * hb * h,
                        ap=[list(psv.ap[0]), [32, hb], [1, 32]],
                    )
                    mk3 = bass.AP(
                        tensor=m_sb.tensor, offset=m_sb.offset,
                        ap=[list(m_sb.ap[0]), [0, hb], [1, 32]],
                    )
                    nc.vector.scalar_tensor_tensor(
                        out=tmp[:], in0=ps3,
                        scalar=b_sb, in1=mk3,
                        op0=mybir.AluOpType.add, op1=mybir.AluOpType.mult,
                    )
                    nc.vector.tensor_reduce(
                        out=red[:, fq, hb * h:hb * h + hb], in_=tmp[:],
                        axis=mybir.AxisListType.X,
                        op=mybir.AluOpType.add,
                    )
                # out DMAs: fq0-2 merged into one deferred 24KB DMA, the
                # last fq alone (8KB) so its receipt is the only tail.
                if fq == FQ - 2:
                    nc.scalar.dma_start(out=out_d[:, 0:FQ - 1],
                                        in_=red[:, 0:FQ - 1])
                elif fq == FQ - 1:
                    nc.scalar.dma_start(out=out_d[:, FQ - 1:FQ],
                                        in_=red[:, FQ - 1:FQ])
    nc.compile()
    return nc


def _prep_inputs(x, weight, bias):
    """Host-side packing into the transposed (mod-8 row, mod-16 col)
    partition layout; bf16 cast.  Returns per-core in_maps."""
    x = np.asarray(x, dtype=np.float32)
    weight = np.asarray(weight, dtype=np.float32)
    bias = np.asarray(bias, dtype=np.float32)

    xpad = np.zeros((B, FULL, FULL), dtype=np.float32)
    xpad[:, PH:PH + H, PW:PW + W] = x[:, 0]
    xpb = xpad.astype(BF16)

    # r = 16*idx + 8*par + dl
    dl = np.arange(8)[:, None, None]
    par = np.arange(2)[None, :, None]
    idx = np.arange(33)[None, None, :]
    r_map = 16 * idx + 8 * par + dl                      # [8, 2, 33]

    w4 = weight.reshape(32, 32, 32, 32)                  # [nr, nc, dr, dc]
    bv = bias.reshape(32, 32)                            # [nr, nc]

    # diag mask: mk[p, x] = (x == p % 32)
    mkv = (np.arange(32)[None, :]
           == (np.arange(128) % 32)[:, None]).astype(BF16)

    in_maps = []
    for k in range(NCORES):
        c_map = (16 * (4 * k + np.arange(M))[:, None]
                 + np.arange(16)[None, :])               # [m, c16]
        # gather -> [b, dl, par, idx, m, c16]
        g = xpb[:, r_map.reshape(8, 2, 33, 1, 1),
                c_map.reshape(1, 1, 1, M, 16)]
        # -> [fq, bi, dl, par, idx, m, c16]
        g = g.reshape(FQ, BFQ, 8, 2, 33, M, 16)
        # -> [fq, par, dl, c16, m, bi, idx]
        g = g.transpose(0, 3, 2, 6, 5, 1, 4)
        g = g.reshape(FQ, 2, 128, M, BFQ, 33)
        xsv = np.ascontiguousarray(g[:, 0])
        x8v = np.ascontiguousarray(g[:, 1]).astype(F8)

        # weights: [nr, j, qr, dl, hc, c16] -> [dl, c16, j, qr, hc, nr]
        wk = w4[:, 4 * k:4 * k + NCL].reshape(32, NCL, 4, 8, 2, 16)
        wk = wk.transpose(3, 5, 1, 2, 4, 0).reshape(128, 1024).astype(BF16)
        # bias: partition 32j + nr_w -> bias[nr_w, 4k+j]
        bk = bv[:, 4 * k:4 * k + NCL].T.reshape(128, 1).astype(BF16)
        wpk = np.ascontiguousarray(np.concatenate([wk, mkv, bk], axis=1))

        in_maps.append({"xs": xsv, "x8": x8v, "wp": wpk})
    return in_maps


def kernel(x, weight, bias):
    global LAST_RESULTS
    from concourse.bass_utils import run_bass_kernel_spmd

    if "nc" not in _CACHE:
        _CACHE["nc"] = _build_program()
    nc = _CACHE["nc"]

    in_maps = _prep_inputs(x, weight, bias)
    res = run_bass_kernel_spmd(
        nc, in_maps, core_ids=list(range(NCORES)), trace=TRACE
    )
    LAST_RESULTS = res

    out = np.empty((B, NKH, NKW), dtype=np.float32)
    for k in range(NCORES):
        # out[32j + nr, fq, bi] -> out[16fq + bi, nr, 4k + j]
        r4 = res.results[k]["out"].reshape(NCL, 32, FQ, BFQ)
        d = r4.transpose(2, 3, 1, 0)            # [fq, bi, nr, j]
        out[:, :, 4 * k:4 * k + NCL] = d.reshape(B, NKH, NCL)
    return out
